# revision 6
# baseline (speedup 1.0000x reference)
"""Trainium2 Bass kernel for nn_MultiScaleDeformableDecoderLayer.

Sharding: data-parallel over batch B=16 across 8 NeuronCores (2 batches
per core), params replicated, no collectives.

Key idea: grid_sample is sparse — instead of streaming the ~436MB of
feature maps, the device computes bilinear corner indices on-chip and
gathers only the needed rows (~4MB/core) with indirect DMA from a
host-packed (row = spatial position, cols = feat||pos channels) layout.
Attention + FFN + heads run fully on-chip per batch.
"""

import numpy as np

# ---- problem constants (hardcoded) ----
C = 256
HEADS = 8
DH = 32
L = 32
B = 16
DFF = 2048
WS = [100, 50, 25, 13]          # W == H per scale
HWS = [w * w for w in WS]
P_PER = 64
EPS = 1e-5
NCORES = 8
BL = 2
ISQ = float(1.0 / np.sqrt(np.float32(DH)))

BASES = []
_off = 0
for _s in range(4):
    _row = []
    for _j in range(BL):
        _row.append(_off)
        _off += HWS[_s]
    BASES.append(_row)
NROWS = _off  # 26588

# matmul dtype config: "f32" or "f32r" per group
CFG = {"proj": "f32", "scores": "f32", "av": "f32", "rank1": "f32"}

_CACHE = {}


def _host_pack(inputs):
    f32 = np.float32
    feats = [np.asarray(inputs[f"feat{i}"], dtype=f32) for i in range(4)]
    poss = [np.asarray(inputs[f"pos{i}"], dtype=f32) for i in range(4)]
    tq = np.asarray(inputs["text_queries"], dtype=f32)
    tpe = np.asarray(inputs["text_pos_embed"], dtype=f32)
    refp = np.asarray(inputs["ref_point"], dtype=f32)
    sq = np.asarray(inputs["sampling_query"], dtype=f32)
    grid0 = np.asarray(inputs["grid0"], dtype=f32)

    def chunkT(w, p=128):
        R, K = w.shape
        assert R % p == 0
        return np.ascontiguousarray(
            np.concatenate([w[i * p:(i + 1) * p] for i in range(R // p)], axis=1)
        ).astype(f32)

    shared = {}
    for pfx in ("sa", "ca"):
        wi = np.asarray(inputs[f"{pfx}_wi"], f32)
        bi = np.asarray(inputs[f"{pfx}_bi"], f32)
        wo = np.asarray(inputs[f"{pfx}_wo"], f32)
        bo = np.asarray(inputs[f"{pfx}_bo"], f32)
        shared[f"{pfx}_wq"] = chunkT(wi[0:C].T.copy())
        shared[f"{pfx}_wk"] = chunkT(wi[C:2 * C].T.copy())
        shared[f"{pfx}_wv"] = chunkT(wi[2 * C:3 * C].T.copy())
        shared[f"{pfx}_wo"] = chunkT(wo.T.copy())
        bq, bk, bv = bi[0:C], bi[C:2 * C], bi[2 * C:3 * C]
        shared[f"{pfx}_bqk"] = np.stack(
            [bq[0:128], bq[128:256], bk[0:128], bk[128:256]], axis=1).astype(f32)
        brow = np.zeros((1, 512), f32)
        brow[0, 0:256] = bv
        brow[0, 256:512] = bo
        shared[f"{pfx}_brow"] = brow
    shared["w1T"] = chunkT(np.asarray(inputs["ffn_w1"], f32).T.copy())
    shared["w2T"] = chunkT(np.asarray(inputs["ffn_w2"], f32).T.copy())
    shared["b1col"] = np.ascontiguousarray(
        np.asarray(inputs["ffn_b1"], f32).reshape(16, 128).T)
    shared["b2row"] = np.asarray(inputs["ffn_b2"], f32).reshape(1, C).copy()
    shared["uw0T"] = chunkT(np.asarray(inputs["upd_w0"], f32).T.copy())
    shared["uw1T"] = chunkT(np.asarray(inputs["upd_w1"], f32).T.copy())
    shared["ub0col"] = np.ascontiguousarray(
        np.asarray(inputs["upd_b0"], f32).reshape(2, 128).T)
    shared["ub1row"] = np.asarray(inputs["upd_b1"], f32).reshape(1, C).copy()
    shared["lnrows"] = np.stack(
        [np.asarray(inputs[k], f32)
         for k in ("ns_g", "ns_b", "n1_g", "n1_b", "n2_g", "n2_b")])

    ow, obs = {}, {}
    for s in (1, 2, 3):
        w = np.asarray(inputs[f"off_w{s}"], f32)
        ow[(s, "x")] = w[0::2].T.copy()
        ow[(s, "y")] = w[1::2].T.copy()
        b = np.asarray(inputs[f"off_b{s}"], f32)
        obs[(s, "x")] = b[0::2]
        obs[(s, "y")] = b[1::2]
    shared["owx_lo"] = chunkT(ow[(1, "x")])
    shared["owy_lo"] = chunkT(ow[(1, "y")])
    shared["owx_hi"] = chunkT(np.concatenate([ow[(2, "x")], ow[(3, "x")]], axis=1))
    shared["owy_hi"] = chunkT(np.concatenate([ow[(2, "y")], ow[(3, "y")]], axis=1))
    z64 = np.zeros(64, f32)
    ob_cols = [np.concatenate([z64, obs[(1, "x")]]),
               np.concatenate([z64, obs[(1, "y")]]),
               np.concatenate([obs[(2, "x")], obs[(3, "x")]]),
               np.concatenate([obs[(2, "y")], obs[(3, "y")]])]
    shared["g0xy"] = np.stack(
        [grid0[:, 0], grid0[:, 0], grid0[:, 1], grid0[:, 1]], axis=1).astype(f32)

    half = np.concatenate([np.zeros(64, np.float64), np.ones(64, np.float64)])
    full = np.ones(128, np.float64)

    in_maps = []
    for k in range(NCORES):
        jb = [2 * k, 2 * k + 1]
        m = dict(shared)
        blocks = []
        for s in range(4):
            for j in range(BL):
                fm = feats[s][jb[j]].reshape(C, HWS[s]).T
                pm = poss[s][jb[j]].reshape(C, HWS[s]).T
                blocks.append(np.concatenate([fm, pm], axis=1))
        m["A"] = np.ascontiguousarray(np.concatenate(blocks, axis=0))
        m["sqT"] = chunkT(sq[jb].T.copy())
        cols = list(ob_cols)
        for arr, coord in ((half, 0), (half, 1), (full, 0), (full, 1)):
            pass
        cols += [half * refp[jb[0], 0], half * refp[jb[1], 0],
                 half * refp[jb[0], 1], half * refp[jb[1], 1],
                 full * refp[jb[0], 0], full * refp[jb[1], 0],
                 full * refp[jb[0], 1], full * refp[jb[1], 1]]
        m["cin"] = np.stack(cols, axis=1).astype(f32)
        m["refT"] = np.ascontiguousarray(refp[jb].T)
        tqt = np.concatenate([tq[:, jb[0], :].T, tq[:, jb[1], :].T], axis=1)
        tpet = np.concatenate([tpe[:, jb[0], :].T, tpe[:, jb[1], :].T], axis=1)
        m["tqT"] = chunkT(tqt)
        m["tpeT"] = chunkT(tpet)
        m["tqrows"] = np.ascontiguousarray(
            np.concatenate([tq[:, jb[0], :], tq[:, jb[1], :]], axis=0))
        in_maps.append(m)
    return in_maps


def _build_nc():
    from contextlib import ExitStack
    from concourse import bass, bacc, tile, mybir
    from concourse.masks import make_identity

    f32 = mybir.dt.float32
    i32 = mybir.dt.int32
    Alu = mybir.AluOpType
    Act = mybir.ActivationFunctionType
    AX = mybir.AxisListType.X

    def mdt(ap, kind):
        if CFG[kind] == "f32r":
            return ap.bitcast(mybir.dt.float32r)
        return ap

    nc = bacc.Bacc("TRN2", target_bir_lowering=False, debug=False)

    di = {}

    def inp(name, shape):
        di[name] = nc.dram_tensor(name, shape, f32, kind="ExternalInput")

    inp("A", [NROWS, 512])
    inp("sqT", [128, 4]); inp("g0xy", [64, 4]); inp("cin", [128, 12])
    inp("refT", [2, 2])
    inp("owx_lo", [128, 128]); inp("owy_lo", [128, 128])
    inp("owx_hi", [128, 256]); inp("owy_hi", [128, 256])
    for pfx in ("sa", "ca"):
        inp(f"{pfx}_wq", [128, 512]); inp(f"{pfx}_wk", [128, 512])
        inp(f"{pfx}_wv", [128, 512]); inp(f"{pfx}_wo", [128, 512])
        inp(f"{pfx}_bqk", [128, 4]); inp(f"{pfx}_brow", [1, 512])
    inp("w1T", [128, 4096]); inp("w2T", [128, 4096])
    inp("b1col", [128, 16]); inp("b2row", [1, C])
    inp("uw0T", [128, 1024]); inp("uw1T", [128, 512])
    inp("ub0col", [128, 2]); inp("ub1row", [1, C])
    inp("lnrows", [6, C])
    inp("tqT", [128, 128]); inp("tpeT", [128, 128]); inp("tqrows", [64, 256])

    to_txt = nc.dram_tensor("text_out", [L, BL, C], f32, kind="ExternalOutput")
    to_ref = nc.dram_tensor("new_ref", [BL, 2], f32, kind="ExternalOutput")
    to_sq = nc.dram_tensor("new_sq", [BL, C], f32, kind="ExternalOutput")

    with tile.TileContext(nc) as tc, ExitStack() as ctx:
        cw = ctx.enter_context(tc.tile_pool(name="cw", bufs=1))
        gp = ctx.enter_context(tc.tile_pool(name="gp", bufs=1))
        wk = ctx.enter_context(tc.tile_pool(name="wk", bufs=1))
        pp = ctx.enter_context(tc.tile_pool(name="pp", bufs=1, space="PSUM"))

        def psum(shape, tag, bufs):
            return pp.tile(shape, f32, tag=tag, bufs=bufs, name=f"ps_{tag}")

        def ld(name):
            t = cw.tile(di[name].shape, f32, tag=f"ld_{name}", name=f"w_{name}")
            nc.sync.dma_start(out=t[:], in_=di[name][:])
            return t

        W = {}
        for name in di:
            if name != "A":
                W[name] = ld(name)

        ident = cw.tile([128, 128], f32, tag="ident", name="ident")
        make_identity(nc, ident[:])
        ones_row = cw.tile([1, 128], f32, tag="ones_row", name="ones_row")
        nc.gpsimd.memset(ones_row[:], 1.0)

        lnb = {}
        for i, nm in enumerate(("ns_g", "ns_b", "n1_g", "n1_b", "n2_g", "n2_b")):
            row = cw.tile([1, C], f32, tag=f"lnrow_{nm}", name=f"lnrow_{nm}")
            nc.sync.dma_start(out=row[:], in_=di["lnrows"][i:i + 1, :])
            t = cw.tile([128, C], f32, tag=f"ln_{nm}", name=f"ln_{nm}")
            nc.gpsimd.partition_broadcast(t[:], row[:])
            lnb[nm] = t

        wc = cw.tile([128, 6], f32, tag="wc", name="wc")
        for col, vals in enumerate(((100., 50.), (25., 13.), (99., 49.),
                                    (24., 12.), (98., 48.), (23., 11.))):
            nc.gpsimd.memset(wc[0:64, col:col + 1], vals[0])
            nc.gpsimd.memset(wc[64:128, col:col + 1], vals[1])
        base_g = {}
        for g, s0, s1 in (("lo", 0, 1), ("hi", 2, 3)):
            t = cw.tile([128, 2], f32, tag=f"base_{g}", name=f"base_{g}")
            nc.gpsimd.memset(t[0:64, 0:1], float(BASES[s0][0]))
            nc.gpsimd.memset(t[0:64, 1:2], float(BASES[s0][1]))
            nc.gpsimd.memset(t[64:128, 0:1], float(BASES[s1][0]))
            nc.gpsimd.memset(t[64:128, 1:2], float(BASES[s1][1]))
            base_g[g] = t

        # ========== Stage A: sampling coordinates ==========
        XY = {}
        for coord in ("x", "y"):
            for g in ("lo", "hi"):
                t = wk.tile([128, 2], f32, tag=f"xy_{coord}_{g}",
                            name=f"xy_{coord}_{g}")
                pst = psum([128, 2], "pp", 4)
                own = W[f"ow{coord}_{g}"]
                if g == "lo":
                    for kc in range(2):
                        nc.tensor.matmul(pst[64:128, :],
                                         lhsT=own[:, kc * 64:(kc + 1) * 64],
                                         rhs=W["sqT"][:, kc * 2:kc * 2 + 2],
                                         start=(kc == 0), stop=(kc == 1))
                    gcol = 0 if coord == "x" else 2
                    nc.sync.dma_start(out=t[0:64, :],
                                      in_=di["g0xy"][:, gcol:gcol + 2])
                    nc.vector.tensor_copy(out=t[64:128, :], in_=pst[64:128, :])
                else:
                    for kc in range(2):
                        nc.tensor.matmul(pst[:],
                                         lhsT=own[:, kc * 128:(kc + 1) * 128],
                                         rhs=W["sqT"][:, kc * 2:kc * 2 + 2],
                                         start=(kc == 0), stop=(kc == 1))
                    nc.vector.tensor_copy(out=t[:], in_=pst[:])
                XY[(coord, g)] = t

        CR = {}
        cin = W["cin"]
        for gi, g in enumerate(("lo", "hi")):
            wv = wc[:, 0 + gi:1 + gi]
            wm1 = wc[:, 2 + gi:3 + gi]
            wm2 = wc[:, 4 + gi:5 + gi]
            for ci, coord in enumerate(("x", "y")):
                ob = cin[:, 2 * gi + ci:2 * gi + ci + 1]
                ref = cin[:, 4 + 4 * gi + 2 * ci: 4 + 4 * gi + 2 * ci + 2]
                p = XY[(coord, g)]
                t1 = wk.tile([128, 2], f32, tag=f"c_t1_{g}{coord}",
                             name=f"c_t1_{g}{coord}")
                nc.vector.scalar_tensor_tensor(out=t1[:], in0=p[:], scalar=ob,
                                               in1=ref[:], op0=Alu.add,
                                               op1=Alu.add)
                nc.vector.tensor_scalar(out=t1[:], in0=t1[:], scalar1=2.0,
                                        scalar2=1.0, op0=Alu.mult,
                                        op1=Alu.subtract)
                nc.vector.tensor_scalar(out=t1[:], in0=t1[:], scalar1=1.0,
                                        scalar2=wv, op0=Alu.add, op1=Alu.mult)
                nc.vector.tensor_scalar(out=t1[:], in0=t1[:], scalar1=1.0,
                                        scalar2=0.5, op0=Alu.subtract,
                                        op1=Alu.mult)
                nc.vector.tensor_scalar(out=t1[:], in0=t1[:], scalar1=0.0,
                                        scalar2=wm1, op0=Alu.max, op1=Alu.min)
                # floor via int cast + correction (valid for trunc or RNE cast)
                xi = wk.tile([128, 2], i32, tag=f"c_xi_{g}{coord}",
                             name=f"c_xi_{g}{coord}")
                nc.vector.tensor_copy(out=xi[:], in_=t1[:])
                xf = wk.tile([128, 2], f32, tag=f"c_xf_{g}{coord}",
                             name=f"c_xf_{g}{coord}")
                nc.vector.tensor_copy(out=xf[:], in_=xi[:])
                gt = wk.tile([128, 2], f32, tag=f"c_gt_{g}{coord}",
                             name=f"c_gt_{g}{coord}")
                nc.vector.tensor_tensor(out=gt[:], in0=xf[:], in1=t1[:],
                                        op=Alu.is_gt)
                x0 = wk.tile([128, 2], f32, tag=f"c_x0_{g}{coord}",
                             name=f"c_x0_{g}{coord}")
                nc.vector.tensor_tensor(out=x0[:], in0=xf[:], in1=gt[:],
                                        op=Alu.subtract)
                nc.vector.tensor_scalar(out=x0[:], in0=x0[:], scalar1=wm2,
                                        scalar2=None, op0=Alu.min)
                wgt = wk.tile([128, 2], f32, tag=f"c_w_{g}{coord}",
                              name=f"c_w_{g}{coord}")
                nc.vector.tensor_tensor(out=wgt[:], in0=t1[:], in1=x0[:],
                                        op=Alu.subtract)
                CR[(g, coord)] = (x0, wgt)

        GIDX, CWTS = {}, {}
        for gi, g in enumerate(("lo", "hi")):
            x0c, wx = CR[(g, "x")]
            y0c, wy = CR[(g, "y")]
            wv = wc[:, 0 + gi:1 + gi]
            ux = wk.tile([128, 2], f32, tag=f"ux_{g}", name=f"ux_{g}")
            uy = wk.tile([128, 2], f32, tag=f"uy_{g}", name=f"uy_{g}")
            nc.vector.tensor_scalar(out=ux[:], in0=wx[:], scalar1=-1.0,
                                    scalar2=1.0, op0=Alu.mult, op1=Alu.add)
            nc.vector.tensor_scalar(out=uy[:], in0=wy[:], scalar1=-1.0,
                                    scalar2=1.0, op0=Alu.mult, op1=Alu.add)
            wts = wk.tile([128, 8], f32, tag=f"wts_{g}", name=f"wts_{g}")
            nc.vector.tensor_tensor(out=wts[:, 0:2], in0=ux[:], in1=uy[:],
                                    op=Alu.mult)
            nc.vector.tensor_tensor(out=wts[:, 2:4], in0=wx[:], in1=uy[:],
                                    op=Alu.mult)
            nc.vector.tensor_tensor(out=wts[:, 4:6], in0=ux[:], in1=wy[:],
                                    op=Alu.mult)
            nc.vector.tensor_tensor(out=wts[:, 6:8], in0=wx[:], in1=wy[:],
                                    op=Alu.mult)
            CWTS[g] = wts
            xb = wk.tile([128, 2], f32, tag=f"xb_{g}", name=f"xb_{g}")
            nc.vector.tensor_tensor(out=xb[:], in0=x0c[:], in1=base_g[g][:],
                                    op=Alu.add)
            idf = wk.tile([128, 4], f32, tag=f"idf_{g}", name=f"idf_{g}")
            nc.vector.scalar_tensor_tensor(out=idf[:, 0:2], in0=y0c[:],
                                           scalar=wv, in1=xb[:],
                                           op0=Alu.mult, op1=Alu.add)
            nc.vector.tensor_scalar(out=idf[:, 2:4], in0=idf[:, 0:2],
                                    scalar1=wv, scalar2=None, op0=Alu.add)
            idx = wk.tile([128, 4], i32, tag=f"idx_{g}", name=f"idx_{g}")
            nc.vector.tensor_copy(out=idx[:], in_=idf[:])
            GIDX[g] = idx

        # ========== Stage B: gathers + bilinear ==========
        S = {}
        for j in range(BL):
            for g in ("lo", "hi"):
                idx = GIDX[g]
                g0 = gp.tile([128, 1024], f32, tag="gth", bufs=3, name="g0")
                g1 = gp.tile([128, 1024], f32, tag="gth", bufs=3, name="g1")
                nc.gpsimd.indirect_dma_start(
                    out=g0[:], out_offset=None, in_=di["A"][:],
                    in_offset=bass.IndirectOffsetOnAxis(ap=idx[:, j:j + 1],
                                                        axis=0))
                nc.gpsimd.indirect_dma_start(
                    out=g1[:], out_offset=None, in_=di["A"][:],
                    in_offset=bass.IndirectOffsetOnAxis(ap=idx[:, 2 + j:3 + j],
                                                        axis=0))
                wts = CWTS[g]
                st = wk.tile([128, 512], f32, tag=f"S_{j}_{g}",
                             name=f"S_{j}_{g}")
                nc.scalar.activation(out=st[:], in_=g0[:, 0:512],
                                     func=Act.Copy, scale=wts[:, 0 + j:1 + j])
                nc.vector.scalar_tensor_tensor(
                    out=st[:], in0=g0[:, 512:1024], scalar=wts[:, 2 + j:3 + j],
                    in1=st[:], op0=Alu.mult, op1=Alu.add)
                nc.vector.scalar_tensor_tensor(
                    out=st[:], in0=g1[:, 0:512], scalar=wts[:, 4 + j:5 + j],
                    in1=st[:], op0=Alu.mult, op1=Alu.add)
                nc.vector.scalar_tensor_tensor(
                    out=st[:], in0=g1[:, 512:1024], scalar=wts[:, 6 + j:7 + j],
                    in1=st[:], op0=Alu.mult, op1=Alu.add)
                S[(j, g)] = st

        # ---------- helpers ----------
        def pe_transpose(dst, dst_off, src_ap, rows=128):
            pt = psum([128, 128], "tp", 2)
            nc.tensor.transpose(out=pt[0:128, 0:rows], in_=src_ap,
                                identity=ident[0:rows, 0:rows])
            nc.scalar.copy(out=dst[:, dst_off:dst_off + rows],
                           in_=pt[0:128, 0:rows])

        def layernorm(x_ap, uname, gname, bname, out_ap, rows=128):
            s1 = wk.tile([128, 1], f32, tag=f"ln_s1_{uname}",
                         name=f"ln_s1_{uname}")
            scr = wk.tile([128, C], f32, tag="ln_scr", name="ln_scr")
            nc.scalar.activation(out=scr[0:rows, :], in_=x_ap,
                                 func=Act.Identity, bias=0.0, scale=1.0,
                                 accum_out=s1[0:rows, :])
            nm = wk.tile([128, 1], f32, tag=f"ln_nm_{uname}",
                         name=f"ln_nm_{uname}")
            nc.vector.tensor_scalar(out=nm[0:rows, :], in0=s1[0:rows, :],
                                    scalar1=-1.0 / C, scalar2=None,
                                    op0=Alu.mult)
            xc = wk.tile([128, C], f32, tag=f"ln_xc_{uname}",
                         name=f"ln_xc_{uname}")
            nc.scalar.activation(out=xc[0:rows, :], in_=x_ap,
                                 func=Act.Identity, bias=nm[0:rows, :],
                                 scale=1.0)
            s2 = wk.tile([128, 1], f32, tag=f"ln_s2_{uname}",
                         name=f"ln_s2_{uname}")
            nc.scalar.activation(out=scr[0:rows, :], in_=xc[0:rows, :],
                                 func=Act.Square, accum_out=s2[0:rows, :])
            var = wk.tile([128, 1], f32, tag=f"ln_var_{uname}",
                          name=f"ln_var_{uname}")
            nc.vector.tensor_scalar(out=var[0:rows, :], in0=s2[0:rows, :],
                                    scalar1=1.0 / C, scalar2=EPS,
                                    op0=Alu.mult, op1=Alu.add)
            nc.scalar.sqrt(out=var[0:rows, :], in_=var[0:rows, :])
            rstd = wk.tile([128, 1], f32, tag=f"ln_rstd_{uname}",
                           name=f"ln_rstd_{uname}")
            nc.vector.reciprocal(out=rstd[0:rows, :], in_=var[0:rows, :])
            nc.scalar.activation(out=xc[0:rows, :], in_=xc[0:rows, :],
                                 func=Act.Copy, scale=rstd[0:rows, :])
            nc.vector.tensor_tensor(out=xc[0:rows, :], in0=xc[0:rows, :],
                                    in1=lnb[gname][0:rows, :], op=Alu.mult)
            nc.vector.tensor_tensor(out=out_ap, in0=xc[0:rows, :],
                                    in1=lnb[bname][0:rows, :], op=Alu.add)

        def attention(j, XT, VT, qT, Lq, pfx):
            wq, wk_, wv, wo = (W[f"{pfx}_wq"], W[f"{pfx}_wk"],
                               W[f"{pfx}_wv"], W[f"{pfx}_wo"])
            bqk = W[f"{pfx}_bqk"]
            brow = W[f"{pfx}_brow"]
            qp, kp = [], []
            for cc in range(2):
                pq_ = psum([128, 256], "pp", 4)
                for kc in range(2):
                    nc.tensor.matmul(
                        pq_[:, 0:Lq],
                        lhsT=mdt(wq[:, kc * 256 + cc * 128:
                                    kc * 256 + (cc + 1) * 128], "proj"),
                        rhs=mdt(qT[kc][:, 0:Lq], "proj"),
                        start=(kc == 0), stop=(kc == 1))
                t = wk.tile([128, 256], f32, tag=f"qp{cc}_{pfx}_{j}",
                            name=f"qp{cc}_{pfx}_{j}")
                nc.scalar.activation(out=t[:, 0:Lq], in_=pq_[:, 0:Lq],
                                     func=Act.Identity,
                                     bias=bqk[:, cc:cc + 1], scale=1.0)
                qp.append(t)
                pk_ = psum([128, 256], "pp", 4)
                for kc in range(2):
                    nc.tensor.matmul(
                        pk_[:],
                        lhsT=mdt(wk_[:, kc * 256 + cc * 128:
                                     kc * 256 + (cc + 1) * 128], "proj"),
                        rhs=mdt(XT[kc][:], "proj"),
                        start=(kc == 0), stop=(kc == 1))
                t2 = wk.tile([128, 256], f32, tag=f"kp{cc}_{pfx}_{j}",
                             name=f"kp{cc}_{pfx}_{j}")
                nc.scalar.activation(out=t2[:], in_=pk_[:], func=Act.Identity,
                                     bias=bqk[:, 2 + cc:3 + cc], scale=1.0)
                kp.append(t2)
            vp = []
            for pc in range(2):
                pv = psum([128, 256], "pp", 4)
                for kc in range(2):
                    nc.tensor.matmul(
                        pv[:],
                        lhsT=mdt(VT[kc][:, pc * 128:(pc + 1) * 128], "proj"),
                        rhs=mdt(wv[:, kc * 256:(kc + 1) * 256], "proj"),
                        start=(kc == 0), stop=False)
                nc.tensor.matmul(pv[:], lhsT=mdt(ones_row[:], "rank1"),
                                 rhs=mdt(brow[:, 0:256], "rank1"),
                                 start=False, stop=True)
                t = wk.tile([128, 264], f32, tag=f"vp{pc}_{pfx}_{j}",
                            name=f"vp{pc}_{pfx}_{j}")
                nc.gpsimd.memset(t[:], 1.0)
                for h in range(8):
                    nc.scalar.copy(out=t[:, 33 * h:33 * h + 32],
                                   in_=pv[:, 32 * h:32 * h + 32])
                vp.append(t)
            npq = (Lq + 127) // 128
            ou = [psum([128, 264], "ou", 2) for _ in range(npq)]
            for h in range(8):
                hc, hr = h // 4, (h % 4) * 32
                esb = []
                for pc in range(2):
                    ep = psum([128, 256], "pp", 4)
                    nc.tensor.matmul(
                        ep[:, 0:Lq],
                        lhsT=mdt(kp[hc][hr:hr + 32, pc * 128:(pc + 1) * 128],
                                 "scores"),
                        rhs=mdt(qp[hc][hr:hr + 32, 0:Lq], "scores"),
                        start=True, stop=True, tile_position=(hr, 0))
                    et = wk.tile([128, 256], f32, tag=f"esb{pc}_{pfx}", bufs=3,
                                 name=f"esb{pc}_{pfx}")
                    nc.scalar.activation(out=et[:, 0:Lq], in_=ep[:, 0:Lq],
                                         func=Act.Exp, scale=ISQ)
                    esb.append(et)
                for qc in range(npq):
                    rows = min(128, Lq - qc * 128)
                    for pc in range(2):
                        nc.tensor.matmul(
                            ou[qc][0:rows, 33 * h:33 * h + 33],
                            lhsT=mdt(esb[pc][:, qc * 128:qc * 128 + rows],
                                     "av"),
                            rhs=mdt(vp[pc][:, 33 * h:33 * h + 33], "av"),
                            start=(pc == 0), stop=(pc == 1))
            osb = []
            for qc in range(npq):
                rows = min(128, Lq - qc * 128)
                rec = wk.tile([128, 8], f32, tag=f"rec_{pfx}_{j}_{qc}",
                              name=f"rec_{pfx}_{j}_{qc}")
                nc.vector.reciprocal(out=rec[0:rows, :],
                                     in_=ou[qc][0:rows, 32::33])
                t = wk.tile([128, 256], f32, tag=f"osb_{pfx}_{j}_{qc}",
                            name=f"osb_{pfx}_{j}_{qc}")
                for h in range(8):
                    nc.vector.tensor_scalar(
                        out=t[0:rows, 32 * h:32 * h + 32],
                        in0=ou[qc][0:rows, 33 * h:33 * h + 32],
                        scalar1=rec[0:rows, h:h + 1], scalar2=None,
                        op0=Alu.mult)
                osb.append(t)
            ot = []
            for cc in range(2):
                t = wk.tile([128, 256], f32, tag=f"ot{cc}_{pfx}_{j}",
                            name=f"ot{cc}_{pfx}_{j}")
                for qc in range(npq):
                    rows = min(128, Lq - qc * 128)
                    pe_transpose(t, qc * 128,
                                 osb[qc][0:rows, cc * 128:(cc + 1) * 128],
                                 rows=rows)
                ot.append(t)
            res = []
            for qc in range(npq):
                rows = min(128, Lq - qc * 128)
                po = psum([128, 256], "pp", 4)
                for cc in range(2):
                    nc.tensor.matmul(
                        po[0:rows, :],
                        lhsT=mdt(ot[cc][:, qc * 128:qc * 128 + rows], "proj"),
                        rhs=mdt(wo[:, cc * 256:(cc + 1) * 256], "proj"),
                        start=(cc == 0), stop=False)
                nc.tensor.matmul(po[0:rows, :],
                                 lhsT=mdt(ones_row[:, 0:rows], "rank1"),
                                 rhs=mdt(brow[:, 256:512], "rank1"),
                                 start=False, stop=True)
                res.append(po)
            return res

        # ========== per-batch ==========
        nrt = wk.tile([2, 2], f32, tag="nrt", name="nrt")
        for j in range(BL):
            ST = [wk.tile([128, 256], f32, tag=f"ST{cc}_{j}",
                          name=f"ST{cc}_{j}") for cc in range(2)]
            PT = [wk.tile([128, 256], f32, tag=f"PT{cc}_{j}",
                          name=f"PT{cc}_{j}") for cc in range(2)]
            for gi, g in enumerate(("lo", "hi")):
                for cc in range(2):
                    pe_transpose(ST[cc], gi * 128,
                                 S[(j, g)][:, cc * 128:(cc + 1) * 128])
                    pe_transpose(PT[cc], gi * 128,
                                 S[(j, g)][:, 256 + cc * 128:
                                           256 + (cc + 1) * 128])
            XT = [wk.tile([128, 256], f32, tag=f"XT{cc}_{j}",
                          name=f"XT{cc}_{j}") for cc in range(2)]
            for cc in range(2):
                nc.vector.tensor_tensor(out=XT[cc][:], in0=ST[cc][:],
                                        in1=PT[cc][:], op=Alu.add)
            sa_ps = attention(j, XT, ST, XT, 256, "sa")
            SN = [wk.tile([128, 256], f32, tag=f"SN{g}_{j}",
                          name=f"SN{g}_{j}") for g in range(2)]
            for gi, g in enumerate(("lo", "hi")):
                rs = wk.tile([128, 256], f32, tag=f"rs_{j}_{g}",
                             name=f"rs_{j}_{g}")
                nc.vector.tensor_tensor(out=rs[:], in0=S[(j, g)][:, 0:256],
                                        in1=sa_ps[gi][:], op=Alu.add)
                layernorm(rs[:], f"ns_{j}_{g}", "ns_g", "ns_b", SN[gi][:])
            SNT = [wk.tile([128, 256], f32, tag=f"SNT{cc}_{j}",
                           name=f"SNT{cc}_{j}") for cc in range(2)]
            for gi in range(2):
                for cc in range(2):
                    pe_transpose(SNT[cc], gi * 128,
                                 SN[gi][:, cc * 128:(cc + 1) * 128])
            X2T = [wk.tile([128, 256], f32, tag=f"X2T{cc}_{j}",
                           name=f"X2T{cc}_{j}") for cc in range(2)]
            for cc in range(2):
                nc.vector.tensor_tensor(out=X2T[cc][:], in0=SNT[cc][:],
                                        in1=PT[cc][:], op=Alu.add)
            QT = [wk.tile([128, 32], f32, tag=f"QT{kc}_{j}",
                          name=f"QT{kc}_{j}") for kc in range(2)]
            for kc in range(2):
                nc.vector.tensor_tensor(
                    out=QT[kc][:],
                    in0=W["tqT"][:, kc * 64 + j * 32:kc * 64 + j * 32 + 32],
                    in1=W["tpeT"][:, kc * 64 + j * 32:kc * 64 + j * 32 + 32],
                    op=Alu.add)
            ca_ps = attention(j, X2T, SNT, QT, 32, "ca")
            TU = wk.tile([32, 256], f32, tag=f"TU_{j}", name=f"TU_{j}")
            rt = wk.tile([32, 256], f32, tag=f"rt_{j}", name=f"rt_{j}")
            nc.vector.tensor_tensor(out=rt[:],
                                    in0=W["tqrows"][j * 32:(j + 1) * 32, :],
                                    in1=ca_ps[0][0:32, :], op=Alu.add)
            layernorm(rt[:], f"n1_{j}", "n1_g", "n1_b", TU[:], rows=32)
            TUT = [wk.tile([128, 32], f32, tag=f"TUT{cc}_{j}",
                           name=f"TUT{cc}_{j}") for cc in range(2)]
            for cc in range(2):
                pt = psum([128, 128], "tp", 2)
                nc.tensor.transpose(out=pt[0:128, 0:32],
                                    in_=TU[0:32, cc * 128:(cc + 1) * 128],
                                    identity=ident[0:32, 0:32])
                nc.scalar.copy(out=TUT[cc][:], in_=pt[0:128, 0:32])
            fo = psum([128, 256], "pp", 4)
            for dc in range(16):
                ph = psum([128, 128], "tp", 2)
                for kc in range(2):
                    nc.tensor.matmul(
                        ph[:, 0:32],
                        lhsT=mdt(W["w1T"][:, kc * 2048 + dc * 128:
                                          kc * 2048 + (dc + 1) * 128], "proj"),
                        rhs=mdt(TUT[kc][:], "proj"),
                        start=(kc == 0), stop=(kc == 1))
                hr = wk.tile([128, 32], f32, tag="h1r", bufs=4, name="h1r")
                nc.scalar.activation(out=hr[:], in_=ph[:, 0:32], func=Act.Relu,
                                     bias=W["b1col"][:, dc:dc + 1], scale=1.0)
                nc.tensor.matmul(fo[0:32, :], lhsT=mdt(hr[:], "proj"),
                                 rhs=mdt(W["w2T"][:, dc * 256:(dc + 1) * 256],
                                         "proj"),
                                 start=(dc == 0), stop=False)
            nc.tensor.matmul(fo[0:32, :], lhsT=mdt(ones_row[:, 0:32], "rank1"),
                             rhs=mdt(W["b2row"][:], "rank1"),
                             start=False, stop=True)
            TO = wk.tile([32, 256], f32, tag=f"TO_{j}", name=f"TO_{j}")
            rt2 = wk.tile([32, 256], f32, tag=f"rt2_{j}", name=f"rt2_{j}")
            nc.vector.tensor_tensor(out=rt2[:], in0=TU[:], in1=fo[0:32, :],
                                    op=Alu.add)
            layernorm(rt2[:], f"n2_{j}", "n2_g", "n2_b", TO[:], rows=32)
            nc.sync.dma_start(out=to_txt[:, j, :], in_=TO[:])
            pooled = []
            for cc in range(2):
                pl = wk.tile([128, 1], f32, tag=f"pl{cc}_{j}",
                             name=f"pl{cc}_{j}")
                nc.vector.tensor_reduce(out=pl[:], in_=TUT[cc][:], axis=AX,
                                        op=Alu.add)
                nc.vector.tensor_scalar(out=pl[:], in0=pl[:],
                                        scalar1=1.0 / 32, scalar2=None,
                                        op0=Alu.mult)
                pooled.append(pl)
            sg = wk.tile([2, 2], f32, tag=f"sg_{j}", name=f"sg_{j}")
            nc.scalar.activation(out=sg[0:2, 0:1], in_=pooled[0][0:2, :],
                                 func=Act.Sigmoid)
            nc.vector.scalar_tensor_tensor(out=sg[0:2, 1:2],
                                           in0=sg[0:2, 0:1], scalar=-0.5,
                                           in1=W["refT"][0:2, j:j + 1],
                                           op0=Alu.add, op1=Alu.add)
            nc.vector.tensor_scalar(out=nrt[0:2, j:j + 1], in0=sg[0:2, 1:2],
                                    scalar1=0.0, scalar2=1.0,
                                    op0=Alu.max, op1=Alu.min)
            hT = []
            for ms in range(2):
                phh = psum([128, 128], "tp", 2)
                for kc in range(4):
                    rhs_ap = (pooled[kc][:] if kc < 2 else
                              W["sqT"][:, (kc - 2) * 2 + j:
                                       (kc - 2) * 2 + j + 1])
                    nc.tensor.matmul(
                        phh[:, 0:1],
                        lhsT=W["uw0T"][:, kc * 256 + ms * 128:
                                       kc * 256 + (ms + 1) * 128],
                        rhs=rhs_ap,
                        start=(kc == 0), stop=(kc == 3))
                ht = wk.tile([128, 1], f32, tag=f"hT{ms}_{j}",
                             name=f"hT{ms}_{j}")
                nc.scalar.activation(out=ht[:], in_=phh[:, 0:1],
                                     func=Act.Relu,
                                     bias=W["ub0col"][:, ms:ms + 1], scale=1.0)
                hT.append(ht)
            nsq = psum([128, 256], "pp", 4)
            for ms in range(2):
                nc.tensor.matmul(nsq[0:1, :], lhsT=hT[ms][:],
                                 rhs=W["uw1T"][:, ms * 256:(ms + 1) * 256],
                                 start=(ms == 0), stop=(ms == 1))
            nsr = wk.tile([1, 256], f32, tag=f"nsr_{j}", name=f"nsr_{j}")
            nc.vector.tensor_tensor(out=nsr[:], in0=nsq[0:1, :],
                                    in1=W["ub1row"][:], op=Alu.add)
            nc.sync.dma_start(out=to_sq[j:j + 1, :], in_=nsr[:])
        nc.sync.dma_start(out=to_ref[:].rearrange("b c -> c b"), in_=nrt[:])

    nc.compile()
    return nc


def _get_nc():
    if "nc" not in _CACHE:
        _CACHE["nc"] = _build_nc()
    return _CACHE["nc"]


def kernel(**inputs):
    from concourse.bass_utils import run_bass_kernel_spmd
    nc = _get_nc()
    in_maps = _host_pack(inputs)
    res = run_bass_kernel_spmd(nc, in_maps, core_ids=list(range(NCORES)))
    outs = res.results
    text_out = np.concatenate([o["text_out"] for o in outs], axis=1)
    new_ref = np.concatenate([o["new_ref"] for o in outs], axis=0)
    new_sq = np.concatenate([o["new_sq"] for o in outs], axis=0)
    return (text_out, new_ref, new_sq)
